# revision 21
# baseline (speedup 1.0000x reference)
"""Trainium2 Bass kernel for nn_Entangle_layer (4-gate entangle layer).

Contract: kernel(state) takes the FULL input state, complex64
(16, 4, 1, 16384, 1), returns the FULL output (16, 4, 16, 16384, 1)
complex64.  Sharding: batch 16 -> 2 per core across 8 cores (pure data
parallel).

Math: each output dc-slice r (4 bits, bit k = gate k's branch) applies,
per block, the product of at most two 2x2 gates:
  blk0: diag(q0, bit0) * diag(q7, bit2)       (diag = 1 on |0>, +-i on |1>)
  blk1: dense(q13, bit0) * diag(q2, bit1)
  blk2: dense(q5, bit1) * diag(q10, bit3)
  blk3: dense(q1, bit2) * dense(q12, bit3)
dense tI: out_e = (s - i d)/2, out_o = (s + i d)/2 (s = e+o, d = e-o);
tZ = same with e/o outputs swapped.  Only 4 distinct variants per block;
each is stored to 4 r-positions.

Layout: one SBUF tile row-dim = state bits n[13:7] (per-block rotated so
the partition-axis gate bit is the partition MSB), free dim = n[6:0] x
(re, im) interleaved f32, both batches side by side ([128, 512] f32).
Loads/stores express the row rotation on the DRAM-side access pattern so
SBUF access patterns stay natural.
"""

import numpy as np
from contextlib import ExitStack

import concourse.bass as bass
import concourse.tile as tile
from concourse import bacc, mybir
from concourse.bass_utils import run_bass_kernel_spmd

F32 = mybir.dt.float32
OP = mybir.AluOpType

N_CORES = 8
BATCH = 16
BPC = BATCH // N_CORES  # batches per core
NBLK = 4
DIM = 16384
ROW = 256  # f32 per DRAM row (128 complex)
NROW = 128  # rows per (batch, blk) slice
CH = BPC * ROW  # free f32 per SBUF tile row (both batches)

# per-block partition rotation k: on-chip p = rotr7(R, k), i.e. the low k
# bits of R become the high k bits of p -> p bit6 = R bit (k-1).
# blk2 needs R1 (q5) at p6 -> k=2.  blk0/1/3 use the natural layout (k=0)
# so their 4 r-copies form large single store DMAs; blk1 handles its
# partition-axis diag (R4) with per-partition mask scalars, blk3 handles its
# partition-axis dense (R5) with slab-staged butterflies.
ROTK = (0, 0, 2, 0)


def _rot_dram(ap1d, k):
    """AP over one 32768-f32 DRAM region walked in on-chip partition order
    for the rot_left-k row layout."""
    hi, lo = NROW >> k, 1 << k
    a = ap1d.rearrange("(hi lo f) -> hi lo f", hi=hi, lo=lo, f=ROW)
    if k:
        a = a.transpose([1, 0, 2])
    return a


def _load_x(nc, x, xpool, blk, tag):
    """Allocate + load one block's [128, 512] tile (both batches, one DMA)."""
    X = xpool.tile([NROW, CH], F32, tag=tag)
    xb = x[:, blk].rearrange("b (p f) -> b p f", p=NROW).transpose([1, 0, 2])
    nc.gpsimd.dma_start(X[:, :].rearrange("p (b f) -> p b f", b=BPC), xb)
    return X


def _emit_blk0(nc, x, y, xpool, vpool, tpool, st_eng, X):
    """blk0: diag(q0 -> p bit6, r-bit0) * diag(q7 -> col half, r-bit2)."""
    R = tpool.tile([NROW, CH], F32, tag="r0")  # R = i*X
    Xg = X[:, :].rearrange("p (g two) -> p g two", two=2)
    Rg = R[:, :].rearrange("p (g two) -> p g two", two=2)
    nc.vector.tensor_scalar_mul(Rg[:, :, 0], Xg[:, :, 1], -1.0)
    nc.vector.tensor_scalar_mul(Rg[:, :, 1], Xg[:, :, 0], 1.0)

    V = vpool.tile([NROW, 4 * CH], F32, tag="v0")
    # views: [p, v, b, h(col half), f]
    Vq = V[:, :].rearrange("p (v b h f) -> p v b h f", v=4, b=BPC, h=2)
    Xh = X[:, :].rearrange("p (b h f) -> p b h f", b=BPC, h=2)
    Rh = R[:, :].rearrange("p (b h f) -> p b h f", b=BPC, h=2)

    def bc(src, nv):  # broadcast [64, b, f] -> [64, nv, b, f]
        return src.unsqueeze(1).to_broadcast([64, nv, BPC, ROW // 2])

    lo, up = slice(0, 64), slice(64, 128)
    # quadrant A (p<64, h=0): all v <- X
    nc.scalar.copy(Vq[lo, :, :, 0, :], bc(Xh[lo, :, 0, :], 4))
    # quadrant B (p>=64, h=0): v{0,2} <- -R, v{1,3} <- +R   (sigma0 = -1,+1)
    nc.vector.tensor_scalar_mul(Vq[up, 0:3:2, :, 0, :], bc(Rh[up, :, 0, :], 2), -1.0)
    nc.scalar.copy(Vq[up, 1:4:2, :, 0, :], bc(Rh[up, :, 0, :], 2))
    # quadrant C (p<64, h=1): v{0,1} <- -R, v{2,3} <- +R    (sigma2)
    nc.vector.tensor_scalar_mul(Vq[lo, 0:2, :, 1, :], bc(Rh[lo, :, 1, :], 2), -1.0)
    nc.scalar.copy(Vq[lo, 2:4, :, 1, :], bc(Rh[lo, :, 1, :], 2))
    # quadrant D (p>=64, h=1): v{0,3} <- -X, v{1,2} <- +X   (-sigma0*sigma2)
    nc.vector.tensor_scalar_mul(Vq[up, 0:4:3, :, 1, :], bc(Xh[up, :, 1, :], 2), -1.0)
    nc.scalar.copy(Vq[up, 1:3, :, 1, :], bc(Xh[up, :, 1, :], 2))

    # stores: v = b0 + 2*b2 -> r = b0 + 4*b2 + 2*b1 + 8*b3; pair over b1 per DMA
    for v in range(4):
        for b in range(BPC):
            src = V[:, v * CH + b * ROW : v * CH + (b + 1) * ROW]
            srcb = src.unsqueeze(1).to_broadcast([NROW, 2, ROW])
            for b3 in range(2):
                base = (v & 1) + 4 * (v >> 1) + 8 * b3
                dst = (
                    y[b, 0, base : base + 3 : 2]
                    .rearrange("r (p f) -> r p f", p=NROW)
                    .transpose([1, 0, 2])
                )
                st_eng().dma_start(dst, srcb)


def _emit_blk1(nc, x, y, xpool, vpool, tpool, st_eng, masks, X):
    """blk1 (natural layout): dense(q13 -> adjacent cplx pairs, r-bit0) then
    diag(q2 -> R4 = p bit4 row classes via per-partition mask scalars,
    r-bit1)."""
    am, bp, bn = masks  # [128,1]: am = (R4==0), bp = +(R4==1), bn = -(R4==1)
    X4 = X[:, :].rearrange("p (g x) -> p g x", x=4)  # [e_re, e_im, o_re, o_im]
    S = tpool.tile([NROW, CH // 2], F32, tag="s1")
    D = tpool.tile([NROW, CH // 2], F32, tag="d1")
    Dh = tpool.tile([NROW, CH // 2], F32, tag="dh1")
    S2 = S[:, :].rearrange("p (g two) -> p g two", two=2)
    D2 = D[:, :].rearrange("p (g two) -> p g two", two=2)
    Dh2 = Dh[:, :].rearrange("p (g two) -> p g two", two=2)
    nc.vector.tensor_tensor(S2, X4[:, :, 0:2], X4[:, :, 2:4], OP.add)
    nc.vector.tensor_tensor(D2, X4[:, :, 0:2], X4[:, :, 2:4], OP.subtract)
    nc.vector.tensor_scalar_mul(Dh[:, :], D[:, :], 0.5)

    # T0 = tI result: e = 0.5*s - i*Dh, o = 0.5*s + i*Dh
    T0 = tpool.tile([NROW, CH], F32, tag="t1")
    T4 = T0[:, :].rearrange("p (g x) -> p g x", x=4)
    stt = nc.vector.scalar_tensor_tensor
    stt(T4[:, :, 0], S2[:, :, 0], 0.5, Dh2[:, :, 1], OP.mult, OP.add)
    stt(T4[:, :, 1], S2[:, :, 1], 0.5, Dh2[:, :, 0], OP.mult, OP.subtract)
    stt(T4[:, :, 2], S2[:, :, 0], 0.5, Dh2[:, :, 1], OP.mult, OP.subtract)
    stt(T4[:, :, 3], S2[:, :, 1], 0.5, Dh2[:, :, 0], OP.mult, OP.add)

    # U0 = i*T0, U1 = i*T1 (T1 = pair-swapped T0), full tiles
    U0 = tpool.tile([NROW, CH], F32, tag="u1a")
    U1 = tpool.tile([NROW, CH], F32, tag="u1b")
    U0g = U0[:, :].rearrange("p (g two) -> p g two", two=2)
    T2 = T0[:, :].rearrange("p (g two) -> p g two", two=2)
    nc.vector.tensor_scalar_mul(U0g[:, :, 0], T2[:, :, 1], -1.0)
    nc.scalar.copy(U0g[:, :, 1], T2[:, :, 0])
    U14 = U1[:, :].rearrange("p (g x) -> p g x", x=4)
    nc.vector.tensor_scalar_mul(U14[:, :, 0], T4[:, :, 3], -1.0)
    nc.scalar.copy(U14[:, :, 1], T4[:, :, 2])
    nc.vector.tensor_scalar_mul(U14[:, :, 2], T4[:, :, 1], -1.0)
    nc.scalar.copy(U14[:, :, 3], T4[:, :, 0])

    # Tm_b0 = T_b0 masked to R4==0 rows (T1 = pair-swap of T0, masked the
    # same way since the mask is row-only)
    Tm0 = tpool.tile([NROW, CH], F32, tag="tm0")
    Tm1 = tpool.tile([NROW, CH], F32, tag="tm1")
    Tm14 = Tm1[:, :].rearrange("p (g x) -> p g x", x=4)
    nc.vector.tensor_scalar(Tm0[:, :], T0[:, :], am[:, :], None, OP.mult)
    nc.vector.tensor_scalar(Tm14[:, :, 0:2], T4[:, :, 2:4], am[:, :], None, OP.mult)
    nc.vector.tensor_scalar(Tm14[:, :, 2:4], T4[:, :, 0:2], am[:, :], None, OP.mult)

    V = vpool.tile([NROW, 4 * CH], F32, tag="v1")
    Vv = V[:, :].rearrange("p (v f) -> p v f", v=4)
    # v = b0 + 2*b1: V_v = Tm_b0 + sigma1 * (U_b0 on R4==1 rows)
    stt(Vv[:, 0, :], U0[:, :], bn[:, :], Tm0[:, :], OP.mult, OP.add)
    stt(Vv[:, 1, :], U1[:, :], bn[:, :], Tm1[:, :], OP.mult, OP.add)
    stt(Vv[:, 2, :], U0[:, :], bp[:, :], Tm0[:, :], OP.mult, OP.add)
    stt(Vv[:, 3, :], U1[:, :], bp[:, :], Tm1[:, :], OP.mult, OP.add)

    # stores: r = v + 4*k3 -> one DMA per (v, b) covering the 4 r-copies
    for v in range(4):
        for b in range(BPC):
            src = V[:, v * CH + b * ROW : v * CH + (b + 1) * ROW]
            srcb = src.unsqueeze(1).to_broadcast([NROW, 4, ROW])
            dst = (
                y[b, 1, v : 16 : 4]
                .rearrange("r (p f) -> r p f", p=NROW)
                .transpose([1, 0, 2])
            )
            st_eng().dma_start(dst, srcb)


def _emit_blk2(nc, x, y, xpool, vpool, tpool, ppool, st_eng, Wt, X):
    """blk2 (natural layout): dense(q5 -> R1 = p bit1 pairs, r-bit1) then
    diag(q10 -> j bit4, r-bit3).

    The R1 partner is p^2 -- not slab-representable, so the butterfly runs on
    the TensorEngine: Sf = A_s @ X (pair averages), Df = A_d @ X (signed pair
    half-differences, sign-flipped on R1=1 rows).  Then M0 = Sf - i*Df (tI
    outputs land in natural rows) and M1 = Sf + i*Df (tZ).
    """
    psS = ppool.tile([NROW, CH], F32, tag="psS")
    psD = ppool.tile([NROW, CH], F32, tag="psD")
    nc.tensor.matmul(psS[:, :], Wt[:, 0:128], X[:, :], start=True, stop=True)
    nc.tensor.matmul(psD[:, :], Wt[:, 128:256], X[:, :], start=True, stop=True)

    # HW: an op may read only one non-scalar input from PSUM -> stage Df in SBUF
    Dsb = tpool.tile([NROW, CH], F32, tag="d2sb")
    nc.vector.tensor_copy(Dsb[:, :], psD[:, :])

    M0 = tpool.tile([NROW, CH], F32, tag="m2a")
    M1 = tpool.tile([NROW, CH], F32, tag="m2b")
    M0g = M0[:, :].rearrange("p (g two) -> p g two", two=2)
    M1g = M1[:, :].rearrange("p (g two) -> p g two", two=2)
    Sg = psS[:, :].rearrange("p (g two) -> p g two", two=2)
    Dg = Dsb[:, :].rearrange("p (g two) -> p g two", two=2)
    nc.vector.tensor_tensor(M0g[:, :, 0], Sg[:, :, 0], Dg[:, :, 1], OP.add)
    nc.vector.tensor_tensor(M0g[:, :, 1], Sg[:, :, 1], Dg[:, :, 0], OP.subtract)
    nc.vector.tensor_tensor(M1g[:, :, 0], Sg[:, :, 0], Dg[:, :, 1], OP.subtract)
    nc.vector.tensor_tensor(M1g[:, :, 1], Sg[:, :, 1], Dg[:, :, 0], OP.add)

    V = vpool.tile([NROW, 4 * CH], F32, tag="v2")
    # j bit4 sub-blocks: [p, v, g(16), h(2), q(8), two]
    Vp = V[:, :].rearrange(
        "p (v g h q two) -> p v g h q two", v=4, g=16, h=2, q=8, two=2
    )
    for v in range(4):
        b1, b3 = v & 1, v >> 1
        sg = -1.0 if b3 == 0 else 1.0
        Msrc = M0 if b1 == 0 else M1
        Mp = Msrc[:, :].rearrange(
            "p (g h q two) -> p g h q two", g=16, h=2, q=8, two=2
        )
        # h=0: straight copy; h=1: multiply by i*sg
        nc.scalar.copy(Vp[:, v, :, 0, :, :], Mp[:, :, 0, :, :])
        nc.vector.tensor_scalar_mul(Vp[:, v, :, 1, :, 0], Mp[:, :, 1, :, 1], -sg)
        nc.vector.tensor_scalar_mul(Vp[:, v, :, 1, :, 1], Mp[:, :, 1, :, 0], sg)

    # stores: r = b0 + 2*b1 + 4*b2 + 8*b3; one DMA per (v, b, b2) covering
    # the contiguous b0-pair
    for v in range(4):
        b1, b3 = v & 1, v >> 1
        for b in range(BPC):
            src = V[:, v * CH + b * ROW : v * CH + (b + 1) * ROW]
            srcb = src.unsqueeze(1).to_broadcast([NROW, 2, ROW])
            for b2 in range(2):
                base = 2 * b1 + 4 * b2 + 8 * b3
                dst = (
                    y[b, 2, base : base + 2]
                    .rearrange("r (p f) -> r p f", p=NROW)
                    .transpose([1, 0, 2])
                )
                st_eng().dma_start(dst, srcb)


def _emit_blk3(nc, x, y, xpool, vpool, tpool, st_eng, X):
    """blk3 (natural layout): dense(q1 -> R5 = p bit5 slab pairs, r-bit2)
    then dense(q12 -> j bit2, r-bit3).

    R5 slab pairs: e-rows {0:32, 64:96}, o-rows {32:64, 96:128}, partner at
    p+32.  o-rows are staged via SBUF->SBUF DMA so each tensor_tensor sees
    equal input base partitions (HW requirement)."""
    # stage o-rows so they sit at the same base partitions as their partners
    O3 = tpool.tile([NROW, CH], F32, tag="o3")
    nc.sync.dma_start(O3[0:32, :], X[32:64, :])
    nc.sync.dma_start(O3[64:96, :], X[96:128, :])

    S1 = tpool.tile([NROW, CH], F32, tag="s3")  # slabs 0:32, 32:64 hold pairs
    D1 = tpool.tile([NROW, CH], F32, tag="d3")
    D1h = tpool.tile([NROW, CH], F32, tag="dh3")
    # pair A: rows 0:32 (+32); pair B: rows 64:96 (+32)
    nc.vector.tensor_tensor(S1[0:32, :], X[0:32, :], O3[0:32, :], OP.add)
    nc.vector.tensor_tensor(S1[32:64, :], X[64:96, :], O3[64:96, :], OP.add)
    nc.vector.tensor_tensor(D1[0:32, :], X[0:32, :], O3[0:32, :], OP.subtract)
    nc.vector.tensor_tensor(D1[32:64, :], X[64:96, :], O3[64:96, :], OP.subtract)
    nc.vector.tensor_scalar_mul(D1h[0:64, :], D1[0:64, :], 0.5)

    # M0 (stage-1 tI): e-out rows {0:32, 64:96}, o-out rows {32:64, 96:128}
    M = tpool.tile([NROW, CH], F32, tag="m3")
    M2 = M[:, :].rearrange("p (g two) -> p g two", two=2)
    S12 = S1[:, :].rearrange("p (g two) -> p g two", two=2)
    D1h2 = D1h[:, :].rearrange("p (g two) -> p g two", two=2)
    stt = nc.vector.scalar_tensor_tensor
    for dst_e, dst_o, src in ((slice(0, 32), slice(32, 64), slice(0, 32)),
                              (slice(64, 96), slice(96, 128), slice(32, 64))):
        stt(M2[dst_e, :, 0], S12[src, :, 0], 0.5, D1h2[src, :, 1], OP.mult, OP.add)
        stt(M2[dst_e, :, 1], S12[src, :, 1], 0.5, D1h2[src, :, 0], OP.mult, OP.subtract)
        stt(M2[dst_o, :, 0], S12[src, :, 0], 0.5, D1h2[src, :, 1], OP.mult, OP.subtract)
        stt(M2[dst_o, :, 1], S12[src, :, 1], 0.5, D1h2[src, :, 0], OP.mult, OP.add)

    # stage 2: dense along columns, partner complex at f_c xor 2 (j groups of 8)
    M8 = M[:, :].rearrange("p (g h x) -> p g h x", g=CH // 8, h=2, x=4)
    S2 = tpool.tile([NROW, CH // 2], F32, tag="s3b")
    D2 = tpool.tile([NROW, CH // 2], F32, tag="d3b")
    D2h = tpool.tile([NROW, CH // 2], F32, tag="dh3b")
    S2v = S2[:, :].rearrange("p (g x) -> p g x", x=4)
    D2v = D2[:, :].rearrange("p (g x) -> p g x", x=4)
    nc.vector.tensor_tensor(S2v, M8[:, :, 0, :], M8[:, :, 1, :], OP.add)
    nc.vector.tensor_tensor(D2v, M8[:, :, 0, :], M8[:, :, 1, :], OP.subtract)
    nc.vector.tensor_scalar_mul(D2h[:, :], D2[:, :], 0.5)

    # b2=1 (stage-1 tZ) swaps rows within each R5 slab pair: materialize
    # slab-swapped copies of S2/D2h once (single-input cross-partition copies)
    S2w = tpool.tile([NROW, CH // 2], F32, tag="s3w")
    D2w = tpool.tile([NROW, CH // 2], F32, tag="dh3w")
    for a, bsl in ((slice(0, 32), slice(32, 64)), (slice(64, 96), slice(96, 128))):
        nc.scalar.copy(S2w[a, :], S2[bsl, :])
        nc.scalar.copy(S2w[bsl, :], S2[a, :])
        nc.scalar.copy(D2w[a, :], D2h[bsl, :])
        nc.scalar.copy(D2w[bsl, :], D2h[a, :])

    # component views [p, g, c(2), two]: re = [..., 0], im = [..., 1]
    V = vpool.tile([NROW, 4 * CH], F32, tag="v3")
    V8 = V[:, :].rearrange(
        "p (v g h c two) -> p v g h c two", v=4, g=CH // 8, h=2, c=2, two=2
    )
    for v in range(4):
        b2, b3 = v & 1, v >> 1
        eh, oh = (0, 1) if b3 == 0 else (1, 0)
        Ssrc, Dsrc = (S2, D2h) if b2 == 0 else (S2w, D2w)
        Sc = Ssrc[:, :].rearrange("p (g c two) -> p g c two", c=2, two=2)
        Dc = Dsrc[:, :].rearrange("p (g c two) -> p g c two", c=2, two=2)
        stt(V8[:, v, :, eh, :, 0], Sc[:, :, :, 0], 0.5, Dc[:, :, :, 1],
            OP.mult, OP.add)
        stt(V8[:, v, :, eh, :, 1], Sc[:, :, :, 1], 0.5, Dc[:, :, :, 0],
            OP.mult, OP.subtract)
        stt(V8[:, v, :, oh, :, 0], Sc[:, :, :, 0], 0.5, Dc[:, :, :, 1],
            OP.mult, OP.subtract)
        stt(V8[:, v, :, oh, :, 1], Sc[:, :, :, 1], 0.5, Dc[:, :, :, 0],
            OP.mult, OP.add)

    # stores: r = {0..3} + 4*b2 + 8*b3 -> one DMA per (v, b) over 4 r-copies
    for v in range(4):
        b2, b3 = v & 1, v >> 1
        base = 4 * b2 + 8 * b3
        for b in range(BPC):
            src = V[:, v * CH + b * ROW : v * CH + (b + 1) * ROW]
            srcb = src.unsqueeze(1).to_broadcast([NROW, 4, ROW])
            dst = (
                y[b, 3, base : base + 4]
                .rearrange("r (p f) -> r p f", p=NROW)
                .transpose([1, 0, 2])
            )
            st_eng().dma_start(dst, srcb)


class _NullEng:
    def __getattr__(self, name):
        return lambda *a, **k: None


class _ComputeOffProxy:
    """Forwards DMA engines; swallows vector/scalar compute ops."""

    def __init__(self, nc):
        self._nc = nc
        self._null = _NullEng()

    def __getattr__(self, name):
        if name in ("vector", "scalar"):
            return self._null
        return getattr(self._nc, name)


def build_program(
    skip_stores=False,
    skip_compute=False,
    blocks=(0, 1, 2, 3),
    store_engines=("sync", "scalar", "gpsimd"),
    bufs=2,
):
    nc = bacc.Bacc("TRN2", target_bir_lowering=False, debug=False)
    x = nc.dram_tensor("x", [BPC, NBLK, 2 * DIM], F32, kind="ExternalInput")
    w = nc.dram_tensor("w", [NROW, 2 * NROW], F32, kind="ExternalInput")
    y = nc.dram_tensor("y", [BPC, NBLK, 16, 2 * DIM], F32, kind="ExternalOutput")
    xa, ya = x.ap(), y.ap()

    st_state = [0]
    null = _NullEng()
    engs = [getattr(nc, e) for e in store_engines]

    def st_eng():
        if skip_stores:
            return null
        st_state[0] = (st_state[0] + 1) % len(engs)
        return engs[st_state[0]]

    cnc = _ComputeOffProxy(nc) if skip_compute else nc
    with tile.TileContext(nc) as tc:
        with ExitStack() as ctx:
            cpool = ctx.enter_context(tc.tile_pool(name="c", bufs=1))
            xpool = ctx.enter_context(tc.tile_pool(name="x", bufs=bufs))
            vpool = ctx.enter_context(tc.tile_pool(name="v", bufs=bufs))
            tpool = ctx.enter_context(tc.tile_pool(name="t", bufs=bufs))
            ppool = ctx.enter_context(tc.tile_pool(name="ps", bufs=2, space="PSUM"))
            # blk2 butterfly weights: [A_s | A_d], loaded once
            Wt = cpool.tile([NROW, 2 * NROW], F32, tag="wt")
            nc.gpsimd.dma_start(Wt[:, :], w.ap()[:, :])
            # per-partition masks over R4 = p bit4 (16-row slabs) for blk1:
            # iota(partition index) -> bit4 -> {am = !bit, bp = +bit, bn = -bit}
            pidx = cpool.tile([NROW, 1], mybir.dt.int32, tag="pidx")
            nc.gpsimd.iota(pidx[:, :], pattern=[[0, 1]], base=0, channel_multiplier=1)
            bit = cpool.tile([NROW, 1], mybir.dt.int32, tag="bit")
            nc.vector.tensor_scalar(
                bit[:, :], pidx[:, :], 4, 1,
                OP.logical_shift_right, OP.bitwise_and,
            )
            am = cpool.tile([NROW, 1], F32, tag="am")
            bp = cpool.tile([NROW, 1], F32, tag="bp")
            bn = cpool.tile([NROW, 1], F32, tag="bn")
            nc.vector.tensor_copy(bp[:, :], bit[:, :])  # int -> f32 cast
            nc.vector.tensor_scalar(am[:, :], bp[:, :], -1.0, 1.0, OP.mult, OP.add)
            nc.vector.tensor_scalar_mul(bn[:, :], bp[:, :], -1.0)
            masks = (am, bp, bn)
            # hoist all input loads so Pool's in-order sequencer issues them
            # before any stores queue behind them
            Xts = {b: _load_x(nc, xa, xpool, b, f"x{b}") for b in blocks}
            for b in blocks:
                if b == 0:
                    _emit_blk0(cnc, xa, ya, xpool, vpool, tpool, st_eng, Xts[0])
                elif b == 1:
                    _emit_blk1(cnc, xa, ya, xpool, vpool, tpool, st_eng, masks, Xts[1])
                elif b == 2:
                    _emit_blk2(cnc, xa, ya, xpool, vpool, tpool, ppool, st_eng, Wt, Xts[2])
                else:
                    _emit_blk3(cnc, xa, ya, xpool, vpool, tpool, st_eng, Xts[3])
    nc.compile()
    return nc


def _butterfly_weights():
    """[A_s | A_d] for blk2: A_s = 0.5 at (p,p) and (p, p^2);
    A_d = 0.5 at (p,p), -0.5 at (p, p^2).  Both symmetric, so usable as
    lhsT directly."""
    q = np.arange(NROW)
    W = np.zeros((NROW, 2 * NROW), dtype=np.float32)
    W[q, q] = 0.5
    W[q, q ^ 2] = 0.5
    W[q, NROW + q] = 0.5
    W[q, NROW + (q ^ 2)] = -0.5
    return W


W_NP = _butterfly_weights()

_cached_nc = None


def _get_nc():
    global _cached_nc
    if _cached_nc is None:
        _cached_nc = build_program()
    return _cached_nc


def _make_runner(nc):
    """Build a cached jitted shard_map executable for the SPMD program,
    mirroring concourse.bass2jax.run_bass_via_pjrt but reusable across
    calls (and creating the donated output buffers on device)."""
    import jax
    import jax.numpy as jnp
    from jax.experimental.shard_map import shard_map
    from jax.sharding import Mesh, PartitionSpec
    from concourse import bass2jax as b2j
    from concourse import mybir as mb

    b2j.install_neuronx_cc_hook()

    partition_name = (
        nc.partition_id_tensor.name if nc.partition_id_tensor else None
    )
    in_names, out_names, out_avals, zero_shapes = [], [], [], []
    for alloc in nc.m.functions[0].allocations:
        if not isinstance(alloc, mb.MemoryLocationSet):
            continue
        name = alloc.memorylocations[0].name
        if alloc.kind == "ExternalInput":
            if name != partition_name:
                in_names.append(name)
        elif alloc.kind == "ExternalOutput":
            shape = tuple(alloc.tensor_shape)
            dtype = mb.dt.np(alloc.dtype)
            out_names.append(name)
            out_avals.append(jax.core.ShapedArray(shape, dtype))
            zero_shapes.append((shape, dtype))
    n_params = len(in_names)
    all_names = list(in_names) + list(out_names)
    if partition_name is not None:
        all_names.append(partition_name)
    all_names = tuple(all_names)

    def _body(*args):
        operands = list(args)
        if partition_name is not None:
            operands.append(b2j.partition_id_tensor())
        outs = b2j._bass_exec_p.bind(
            *operands,
            out_avals=tuple(out_avals),
            in_names=all_names,
            out_names=tuple(out_names),
            lowering_input_output_aliases=(),
            sim_require_finite=True,
            sim_require_nnan=True,
            nc=nc,
        )
        return tuple(outs)

    devices = jax.devices()[:N_CORES]
    mesh = Mesh(np.asarray(devices), ("core",))
    n_outs = len(out_names)
    jitted = jax.jit(
        shard_map(
            _body,
            mesh=mesh,
            in_specs=(PartitionSpec("core"),) * (n_params + n_outs),
            out_specs=(PartitionSpec("core"),) * n_outs,
            check_rep=False,
        ),
        donate_argnums=tuple(range(n_params, n_params + n_outs)),
        keep_unused=True,
    )

    from jax.sharding import NamedSharding

    shardings = tuple(
        NamedSharding(mesh, PartitionSpec("core")) for _ in zero_shapes
    )

    def _mk_zeros():
        return tuple(
            jnp.zeros((N_CORES * s[0], *s[1:]), d) for s, d in zero_shapes
        )

    zeros_fn = jax.jit(_mk_zeros, out_shardings=shardings)
    return jitted, in_names, zeros_fn


_cached_runner = None


def _run_fast(xf):
    """Cached-executable path: one PJRT compile per process."""
    global _cached_runner
    if _cached_runner is None:
        _cached_runner = _make_runner(_get_nc())
    jitted, in_names, zeros_fn = _cached_runner
    feeds = {"x": xf, "w": np.tile(W_NP, (N_CORES, 1))}
    outs = jitted(*[feeds[n] for n in in_names], *zeros_fn())
    return np.asarray(outs[0])  # (BATCH, NBLK, 16, 2*DIM) f32


def _run_spmd(xf):
    """Reference path via run_bass_kernel_spmd."""
    nc = _get_nc()
    in_maps = [
        {"x": xf[c * BPC : (c + 1) * BPC], "w": W_NP} for c in range(N_CORES)
    ]
    res = run_bass_kernel_spmd(nc, in_maps, list(range(N_CORES)))
    return np.concatenate(
        [np.asarray(res.results[c]["y"]) for c in range(N_CORES)], axis=0
    )


def kernel(state):
    state = np.ascontiguousarray(np.asarray(state))
    assert state.shape == (BATCH, NBLK, 1, DIM, 1) and state.dtype == np.complex64
    xf = state.view(np.float32).reshape(BATCH, NBLK, 2 * DIM)
    try:
        full = _run_fast(xf)
    except Exception:
        full = _run_spmd(xf)
    out = full.reshape(BATCH, NBLK, 16, DIM, 2).view(np.complex64)
    return out  # (16, 4, 16, 16384, 1) complex64
